# revision 54
# baseline (speedup 1.0000x reference)
"""Trainium2 Bass kernel: multi-head attention block (DiyTransformer).

Full-input contract: kernel(**inputs) takes the unsharded inputs and returns
the full [2, 2048, 1024] output. Internally shards 16 heads across 8
NeuronCores (2 heads = one 128-wide feature slice per core).

Math (reference):
  q = x @ wq.T + bq ; k = x @ wk.T + bk ; v = x @ wv.T + bv   (per-head split)
  out_h = softmax(q_h k_h^T / 8) v_h ;  y = concat(out_h) @ wo.T + bo

Performance structure (defaults):
  - q/k projections in fp8e4 DoubleRow (2 contraction tiles per matmul,
    weights prescaled by 2^13 / 2^10 into fp8 range, descaled at the psum
    evacuation); x is fed both as bf16 (v projection) and fp8 (q/k).
  - exp: 14/16 score-chunk units on ACT (2048-wide calls), last 2 units on
    the DVE as a Schraudolph bit-trick exp (round(x*128/ln2+B) -> int16 ==
    bf16 bits), because ACT is the per-block bottleneck engine.
  - score matmuls of the 2 heads are PE row-tiled (partition bases 0/64)
    into disjoint quadrants -> concurrent (measured ~31us win).
  - inputs one packed DMA per weight + 8 seq-range xT DMAs so projections
    start as soon as the first slice lands; output DMAs merged [128,1024]
    and issued from the gpsimd queue.

Simplifications used here:
  - k bias: adds a per-query constant to every logit in a softmax row ->
    cancels exactly; dropped.
  - v bias: softmax rows sum to 1, so attn @ (v + bv) = attn @ v + bv.
    The bv term is folded into a host-side constant bo_eff = bo + bv @ wo.T.
  - 1/8 scale folded into wq and bq on the host.
  - scores are computed transposed (scoresT[k_pos, q] = k @ qT), so softmax's
    sum runs along the PSUM partition dim. A ones-column appended to v makes
    the PV matmul emit the denominator for free (row 64 of the PV psum), and
    no PE transposes are needed anywhere in the pipeline. (Engines can only
    address 32-aligned partition bases, and the custom-DVE reciprocal drops
    nonzero input bases on HW, so the denominator is staged to partition 0
    before the reciprocal — both heads merged into one staging tile.)

v2 structure changes vs the first working version:
  - host pre-packs xT/wq/wk/wv into SBUF-layout order so each input is ONE
    contiguous DMA (xT: 8 seq-range DMAs so Q/K projection of seq block jj
    can start as soon as its slice lands, instead of after the full 8MB).
  - output DMAs are [128,1024] (2 psum banks staged into one tile) and are
    issued from the gpsimd (Pool) queue, off the busy SP queue.
  - denominator: ones column first -> no den staging copies; reciprocal runs
    directly on psum row 0; one broadcast DMA per block for both heads.
  - v-projection evacuation: one 3D-AP copy per 128-seq chunk (both heads).
  - a dummy 1-wide exp early in the program pulls the ACT table load into
    the input-DMA window.
"""

import sys

sys.path.insert(0, "/opt/trn_rl_repo")

import zlib

import numpy as np
import ml_dtypes

# The axon terminal caches compiled executables by module name + I/O
# signature only (the BIR payload in backend_config is not in the key), so a
# changed kernel with unchanged tensor shapes silently reuses the stale
# executable. Bust it by adding a dummy input whose shape encodes a hash of
# this file's source.
with open(__file__, "rb") as _f:
    _VTAG = (zlib.crc32(_f.read()) % 4093) + 3

D = 1024          # embed dim
NH = 16           # total heads
DH = 64           # head dim
NB = 2            # batch
S = 2048          # seq len
M = NB * S        # 4096 flattened rows
N_CORES = 8
HPC = 2           # heads per core
FS = HPC * DH     # 128 per-core feature slice
DCH = D // 128    # 8 contraction chunks
SCALE = 1.0 / np.sqrt(DH)

BF16 = ml_dtypes.bfloat16

_compiled = None  # (nc, module) cache


QK_SHIFT = 8192.0    # 2**13, q-weight prescale for fp8 (q includes 1/8 scale)
K_SHIFT = 1024.0     # 2**10, k-weight prescale for fp8


def _build(repeat=1, dve_exp=2, interleave=False, defer_po=False,
           recip_direct=False, skip_out=False, serial_scores=False,
           fp8_qk=True, x8_onchip=False, vtag_extra=0):
    import concourse.bass as bass
    import concourse.tile as tile
    from concourse import bacc, mybir

    f32 = mybir.dt.float32
    bf16 = mybir.dt.bfloat16
    f8e4 = mybir.dt.float8e4

    nc = bacc.Bacc("TRN2", target_bir_lowering=False, debug=False,
                   num_devices=N_CORES)

    # all in SBUF-ready layouts (host pre-packed)
    xT_d = nc.dram_tensor("xT", [128, DCH * M], bf16, kind="ExternalInput").ap()
    qk_dt = f8e4 if fp8_qk else bf16
    if fp8_qk and not x8_onchip:
        x8_d = nc.dram_tensor("x8", [128, DCH * M], f8e4,
                              kind="ExternalInput").ap()
    wq_d = nc.dram_tensor("wqT", [128, D], qk_dt, kind="ExternalInput").ap()
    wk_d = nc.dram_tensor("wkT", [128, D], qk_dt, kind="ExternalInput").ap()
    wv_d = nc.dram_tensor("wvT", [128, D], bf16, kind="ExternalInput").ap()
    wo_d = nc.dram_tensor("woT", [FS, D], bf16, kind="ExternalInput").ap()
    bq_d = nc.dram_tensor("bq", [FS, 1], f32, kind="ExternalInput").ap()
    nc.dram_tensor("vtag", [1, _VTAG + vtag_extra + (repeat - 1) * 4096], f32,
                   kind="ExternalInput")
    out_d = nc.dram_tensor("out", [M, D], f32, kind="ExternalOutput").ap()

    Exp = mybir.ActivationFunctionType.Exp

    def ap3(t_ap, extra_off, dims):
        return bass.AP(t_ap.tensor, t_ap.offset + extra_off,
                       [list(t_ap.ap[0])] + [list(d) for d in dims])

    with tile.TileContext(nc) as tc:
        with (
            tc.tile_pool(name="persist", bufs=1) as persist,
            tc.tile_pool(name="stage", bufs=3 if fp8_qk else 4) as stage,
            tc.tile_pool(name="exp", bufs=2) as exp_pool,
            tc.tile_pool(name="oT", bufs=2) as oT_pool,
            tc.tile_pool(name="smalls", bufs=1 if fp8_qk else 2) as smalls,
            tc.tile_pool(name="ps_s0", bufs=1, space="PSUM") as ps_s0,
            tc.tile_pool(name="ps_s1", bufs=1, space="PSUM") as ps_s1,
            tc.tile_pool(name="ps_pv", bufs=2, space="PSUM") as ps_pv,
        ):
            for _rep in range(repeat):
                # ---- load inputs to SBUF (weights first; xT by seq range) ----
                wq = persist.tile([128, D], qk_dt, tag="wq")
                wk = persist.tile([128, D], qk_dt, tag="wk")
                wv = persist.tile([128, D], bf16, tag="wv")
                wo = persist.tile([128, D], bf16, tag="wo")
                bq = persist.tile([FS, 1], f32, tag="bq")
                xT = persist.tile([128, DCH * M], bf16, tag="xT")  # [d-chunk|seq]
                xT_all = xT[:, :]
                if fp8_qk:
                    x8 = persist.tile([128, DCH * M], f8e4, tag="x8")
                    x8_all = x8[:, :]

                Copy = mybir.ActivationFunctionType.Copy

                def xT_dma(jj):
                    off = jj * 512
                    if fp8_qk and not x8_onchip:
                        nc.sync.dma_start(
                            ap3(x8_all, off, [[M, DCH], [1, 512]]),
                            ap3(x8_d, off, [[M, DCH], [1, 512]]))
                    nc.sync.dma_start(
                        ap3(xT_all, off, [[M, DCH], [1, 512]]),
                        ap3(xT_d, off, [[M, DCH], [1, 512]]))
                    if fp8_qk and x8_onchip:
                        # ACT is idle during the projection phase: convert
                        # this seq-range of xT to fp8 on-chip instead of
                        # DMAing a second 4MB copy of x from HBM.
                        nc.scalar.activation(
                            ap3(x8_all, off, [[M, DCH], [1, 512]]),
                            ap3(xT_all, off, [[M, DCH], [1, 512]]), Copy)

                nc.sync.dma_start(bq[:, :], bq_d[:, :])
                nc.sync.dma_start(wq[:, :], wq_d[:, :])
                xT_dma(0)
                nc.sync.dma_start(wk[:, :], wk_d[:, :])
                nc.sync.dma_start(wv[:, :], wv_d[:, :])
                xT_dma(1)
                nc.sync.dma_start(wo[:, :], wo_d[:, :])

                # warm the exp table while DMAs run
                warm = smalls.tile([1, 1], f32, tag="warm")
                nc.vector.memset(warm[:, :], 0.0)
                nc.scalar.activation(warm[:, :], warm[:, :], Exp)

                for jj in range(2, 8):
                    xT_dma(jj)

                # ---- projections ----
                qT = persist.tile([128, M], bf16, tag="qT")   # [feat, seq]
                kT = persist.tile([128, M], bf16, tag="kT")
                # v natural layout + ones column: slot(h,c) = h*32+c, 65 wide
                vv = persist.tile([128, HPC * 32 * 65], bf16, tag="v")
                nc.vector.memset(vv[:, :], 1.0)

                DR = mybir.MatmulPerfMode.DoubleRow

                def qk_matmuls(ps_out, w, jj):
                    if fp8_qk:
                        # DoubleRow: two 128-row k-tiles per matmul; both the
                        # packed weight layout [p, d*128+f] and the x8 layout
                        # [p, d*M+s] already place tile pairs at the right
                        # free-dim strides.
                        for d2 in range(DCH // 2):
                            nc.tensor.matmul(
                                ps_out,
                                ap3(w[:, :], d2 * 256, [[128, 2], [1, 128]]),
                                ap3(x8_all, 2 * d2 * M + jj * 512,
                                    [[M, 2], [1, 512]]),
                                start=(d2 == 0), stop=(d2 == DCH // 2 - 1),
                                perf_mode=DR)
                    else:
                        for d in range(DCH):
                            nc.tensor.matmul(
                                ps_out, w[:, d * 128:(d + 1) * 128],
                                xT[:, d * M + jj * 512: d * M + (jj + 1) * 512],
                                start=(d == 0), stop=(d == DCH - 1))

                def proj_block(jj):
                    qs = slice(jj * 512, (jj + 1) * 512)
                    pq = ps_pv.tile([128, 512], f32, tag="pv")
                    qk_matmuls(pq[:, :], wq, jj)
                    if fp8_qk:
                        nc.vector.tensor_scalar(
                            qT[:, qs], pq[:, :], 1.0 / QK_SHIFT, bq[:, 0:1],
                            mybir.AluOpType.mult, mybir.AluOpType.add)
                    else:
                        nc.vector.tensor_scalar_add(qT[:, qs], pq[:, :],
                                                    bq[:, 0:1])
                    pk = ps_pv.tile([128, 512], f32, tag="pv")
                    qk_matmuls(pk[:, :], wk, jj)
                    if fp8_qk:
                        nc.vector.tensor_scalar_mul(kT[:, qs], pk[:, :],
                                                    1.0 / K_SHIFT)
                    else:
                        nc.vector.tensor_copy(kT[:, qs], pk[:, :])

                    for c in range(jj * 4, jj * 4 + 4):       # v over seq chunks
                        pvreg = ps_pv.tile([128, 512], f32, tag="pv")
                        pv_ = pvreg[:, 0:128]
                        for d in range(DCH):
                            nc.tensor.matmul(pv_, xT[:, d * M + c * 128: d * M + (c + 1) * 128],
                                             wv[:, d * 128:(d + 1) * 128],
                                             start=(d == 0), stop=(d == DCH - 1))
                        # both heads' 64-wide slices in one 3D-AP copy
                        dst = ap3(vv[:, :], c * 65,
                                  [[32 * 65, HPC], [1, 64]])
                        src = ap3(pvreg[:, :], 0, [[64, HPC], [1, 64]])
                        nc.vector.tensor_copy(dst, src)

                # ---- attention + output projection ----
                # Score psum groups: chunks x 2 heads interleaved; the two
                # heads' K=64 matmuls are emitted adjacently with different
                # partition bases (0 / 64) so they row-tile into disjoint PE
                # quadrants and different PSUM banks, running concurrently.
                # exp: ACT does 14/16 chunk-units in up-to-2048-wide calls;
                # the last 2 units run on the DVE as a Schraudolph bit-trick
                # exp (round(x*128/ln2 + B) written as int16 = bf16 bits,
                # ~1.8% rms on these logits), freeing ACT, the per-block
                # bottleneck engine.
                if dve_exp == 4:
                    GROUPS = [(2, "s0", "A"), (1, "s1", "A")] * 4 + \
                             [(2, "s0", "D"), (1, "s1", "D"), (1, "s1", "D")]
                elif dve_exp == 3:
                    # s1-tail: the block ends on s1 groups so the next
                    # block's first (s0) score matmuls don't wait on this
                    # block's last exp.
                    GROUPS = [(2, "s0", "A"), (1, "s1", "A")] * 4 + \
                             [(1, "s0", "A"), (1, "s0", "D"),
                              (1, "s1", "D"), (1, "s1", "D")]
                elif dve_exp == 2:
                    GROUPS = [(2, "s0", "A"), (1, "s1", "A")] * 4 + \
                             [(2, "s0", "A"), (1, "s1", "D"), (1, "s0", "D")]
                else:
                    GROUPS = [(2, "s0", "A"), (1, "s1", "A")] * 5 + \
                             [(1, "s0", "A")]
                EXP_A = 184.66496543257542        # 2**7 / ln(2)
                EXP_B = 16248.75                  # 127*2**7 - c, fitted
                i16 = mybir.dt.int16

                def out_proj(oT, q0):
                    # output projection for these 512 seq rows (4 x 128)
                    for t in range(4):
                        sb = q0 + t * 128
                        oc = stage.tile([128, 1024], f32, tag="oc")
                        for half in range(1 if skip_out else 2):
                            po = ps_pv.tile([128, 512], f32, tag="pv")
                            nc.tensor.matmul(po[:, :], oT[:, t * 128:(t + 1) * 128],
                                             wo[:, half * 512:(half + 1) * 512],
                                             start=True, stop=True)
                            nc.vector.tensor_copy(
                                oc[:, half * 512:(half + 1) * 512], po[:, :])
                        nc.gpsimd.dma_start(out_d[sb:sb + 128, :], oc[:, :])

                pending = []

                def attn_block(n, j):
                        q0 = n * S + j * 512
                        # exp lands in two half-block tiles (chunks 0-7 /
                        # 8-15); PV consumes heads interleaved per chunk so
                        # each half releases as soon as both heads pass it.
                        ets = [exp_pool.tile([128, HPC * 8 * 512], bf16,
                                             tag="exp", name=f"et{_i}")
                               for _i in range(2)]
                        c = 0
                        for cnt, pool_name, eng in GROUPS:
                            pool = ps_s0 if pool_name == "s0" else ps_s1
                            ps = pool.tile([128, cnt * HPC * 512], f32, tag=pool_name)
                            for i in range(cnt):
                                k0 = n * S + (c + i) * 128
                                for h in range(HPC):
                                    # serial_scores: TIMING PROBE ONLY (wrong
                                    # math) - both heads from partition base 0
                                    # so no PE row-tiling concurrency possible
                                    hp = slice(0, DH) if serial_scores else \
                                        slice(h * DH, (h + 1) * DH)
                                    nc.tensor.matmul(
                                        ps[:, (i * HPC + h) * 512:(i * HPC + h + 1) * 512],
                                        kT[hp, k0:k0 + 128],
                                        qT[hp, q0:q0 + 512],
                                        start=True, stop=True)
                            et = ets[c // 8]
                            e0 = (c % 8) * HPC * 512
                            ew = cnt * HPC * 512
                            if eng == "A":
                                nc.scalar.activation(
                                    et[:, e0:e0 + ew], ps[:, :], Exp)
                            else:
                                nc.vector.tensor_scalar(
                                    et[:, e0:e0 + ew].bitcast(i16), ps[:, :],
                                    EXP_A, EXP_B,
                                    mybir.AluOpType.mult, mybir.AluOpType.add)
                            c += cnt
                        # deferred out-proj of the previous block lands here:
                        # its matmuls fill the PE while this block's PV waits
                        # on exp, and the den->recip->bc chain it depended on
                        # has long resolved.
                        while pending:
                            out_proj(*pending.pop(0))
                        oT = oT_pool.tile([128, 512], bf16, tag="oT")
                        det = None if recip_direct else \
                            smalls.tile([1, HPC * 512], f32, tag="den")
                        recip2 = smalls.tile([1, HPC * 512], f32, tag="recip")
                        # PV: heads interleaved per chunk; row 64 = denom
                        pvs = [ps_pv.tile([128, 512], f32, tag="pv",
                                          name=f"pv{_h}")
                               for _h in range(HPC)]
                        for c2 in range(16):
                            et = ets[c2 // 8]
                            e0 = ((c2 % 8) * HPC) * 512
                            for h in range(HPC):
                                vs = (h * 32 + n * 16 + c2) * 65
                                nc.tensor.matmul(
                                    pvs[h][0:65, :],
                                    vv[:, vs:vs + 65],
                                    et[:, e0 + h * 512:e0 + (h + 1) * 512],
                                    start=(c2 == 0), stop=(c2 == 15))
                        for h in range(HPC):
                            if recip_direct:
                                nc.vector.reciprocal_approx_fast(
                                    recip2[:, h * 512:(h + 1) * 512],
                                    pvs[h][64:65, :])
                            else:
                                # stage denominator row to partition 0 (custom
                                # DVE ops drop the input base_partition on HW)
                                nc.vector.tensor_copy(
                                    det[:, h * 512:(h + 1) * 512],
                                    pvs[h][64:65, :])
                        if not recip_direct:
                            nc.vector.reciprocal_approx_fast(recip2[:, :],
                                                             det[:, :])
                        bc2 = smalls.tile([64, HPC * 512], f32, tag="bc")
                        rap = recip2[:, :]
                        nc.sync.dma_start(bc2[:, :], bass.AP(
                            rap.tensor, rap.offset,
                            [[rap.ap[0][0], 1], [0, 64], [1, HPC * 512]]))
                        for h in range(HPC):
                            hp = slice(h * DH, (h + 1) * DH)
                            nc.vector.tensor_mul(oT[hp, :], pvs[h][0:64, :],
                                                 bc2[:, h * 512:(h + 1) * 512])
                        if defer_po:
                            pending.append((oT, q0))
                        else:
                            out_proj(oT, q0)

                if interleave:
                    for jj in range(4):
                        proj_block(jj)
                    attn_block(0, 0)
                    attn_block(0, 1)
                    proj_block(4)
                    proj_block(5)
                    attn_block(0, 2)
                    proj_block(6)
                    proj_block(7)
                    attn_block(0, 3)
                    for j in range(4):
                        attn_block(1, j)
                else:
                    for jj in range(8):
                        proj_block(jj)
                    for n in range(NB):
                        for j in range(4):
                            attn_block(n, j)
                while pending:
                    out_proj(*pending.pop(0))

    nc.compile()
    return nc


def _get_compiled():
    global _compiled
    if _compiled is None:
        _compiled = _build()
    return _compiled


def _pack_rows(a):
    """[D, N] -> [128, DCH*N] chunk-major: out[p, d*N+c] = a[d*128+p, c]."""
    Drows, N = a.shape
    ch = Drows // 128
    return np.ascontiguousarray(
        a.reshape(ch, 128, N).transpose(1, 0, 2).reshape(128, ch * N))


def _prep_in_maps(x, wq, bq, wk, wv, wo, fp8_qk=None, x8_onchip=None):
    import inspect
    if fp8_qk is None:
        fp8_qk = inspect.signature(_build).parameters["fp8_qk"].default
    if x8_onchip is None:
        x8_onchip = inspect.signature(_build).parameters["x8_onchip"].default
    F8 = ml_dtypes.float8_e4m3
    xTf = x.reshape(M, D).T
    xT = _pack_rows(xTf.astype(BF16))
    x8 = _pack_rows(np.clip(xTf, -240, 240).astype(F8)) \
        if (fp8_qk and not x8_onchip) else None
    maps = []
    for i in range(N_CORES):
        rs = slice(i * FS, (i + 1) * FS)
        if fp8_qk:
            wqT = _pack_rows((wq[rs, :] * (SCALE * QK_SHIFT)).T.astype(F8))
            wkT = _pack_rows((wk[rs, :] * K_SHIFT).T.astype(F8))
        else:
            wqT = _pack_rows((wq[rs, :] * SCALE).T.astype(BF16))
            wkT = _pack_rows(wk[rs, :].T.astype(BF16))
        m = {
            "xT": xT,
            "wqT": wqT,
            "wkT": wkT,
            "wvT": _pack_rows(wv[rs, :].T.astype(BF16)),
            "woT": np.ascontiguousarray(wo[:, rs].T).astype(BF16),
            "bq": (bq[rs] * SCALE).astype(np.float32).reshape(FS, 1),
            "vtag": np.zeros((1, _VTAG), np.float32),
        }
        if fp8_qk and x8 is not None:
            m["x8"] = x8
        maps.append(m)
    return maps


def kernel(x, wq, bq, wk, bk, wv, bv, wo, bo, _want_results=False, _trace=False):
    from concourse.bass_utils import run_bass_kernel_spmd

    x = np.asarray(x, dtype=np.float32)
    wq = np.asarray(wq, dtype=np.float32)
    bq = np.asarray(bq, dtype=np.float32)
    wk = np.asarray(wk, dtype=np.float32)
    wv = np.asarray(wv, dtype=np.float32)
    wo = np.asarray(wo, dtype=np.float32)
    bv = np.asarray(bv, dtype=np.float32)
    bo = np.asarray(bo, dtype=np.float32)

    nc = _get_compiled()
    import inspect
    _fp8_default = inspect.signature(_build).parameters["fp8_qk"].default
    in_maps = _prep_in_maps(x, wq, bq, wk, wv, wo, fp8_qk=_fp8_default)
    res = None
    for attempt in range(3):
        try:
            res = run_bass_kernel_spmd(nc, in_maps, list(range(N_CORES)),
                                       trace=_trace)
            break
        except Exception:
            # the shared device occasionally reports
            # NRT_EXEC_UNIT_UNRECOVERABLE transiently; back off and retry
            if attempt == 2:
                raise
            import time as _time
            _time.sleep(15)

    acc = np.zeros((M, D), dtype=np.float32)
    for i in range(N_CORES):
        acc += res.results[i]["out"]
    acc += bo + bv @ wo.T
    out = acc.reshape(NB, S, D)
    if _want_results:
        return out, res
    return out
